# revision 59
# baseline (speedup 1.0000x reference)
"""Multi-head causal self-attention (B=1, S=4096, D=1024, H=16) on 8 TRN2
NeuronCores, tensor-parallel over heads (2 heads per core).

Engine plan (per core, modeled by TimelineSim):
  - PE: QKV projection bf16 (Q^T/K^T weight-stationary -> [feat, s]; V
    X-stationary -> [s, feat], so V needs no transpose), scores QK^T via
    fp8e4 DoubleRow (0.5 cyc/row; chunks 0-2 run bf16 to skip the fp8
    rearrange latency at startup), causal mask added in-PSUM by a ramp
    matmul (-1e30*max(t-s,0) = A^T B, so exp() of masked entries is 0),
    AV with P^T stationary and [V|1] moving (N=65/pass; the denominator
    rides along as column 64), ot [s,f] -> ot^T via PE transpose into a
    dead AV-psum region, output projection bf16.
  - ACT: the softmax exp only (scale=1/8 folded in), one inst per t-tile;
    at the tail it also takes half the output-projection copies.
  - DVE: all psum->sbuf copies (GPSIMD cannot touch PSUM), reciprocal +
    per-partition divide (the [s, f] layout makes the denominator a
    per-partition scalar).
  - SP: all DMAs. fp8 Q/K reach the DoubleRow [32, 2, *] layout via 4
    sbuf->sbuf transfers per tensor, deferred off the startup queue.

PSUM (8 banks, one accumulation group per 2KB zero-region at a time):
  scores pool 2x[128, 2, 512] (4) | AV 2x[128, 2, 2, 128] (2) | QKV (1) |
  outproj (1).

Schedule: one merged software-pipelined stream over (chunk j, t-tile tt):
scores -> exp -> (lag 2) AV, with QKV(j+1) pieces and outproj(j-1) pieces
interleaved between t-tiles, divisions per stile-pair as their AV closes,
PE warmup matmuls during the initial DMAs (pstate ramp), and a split
last-chunk epilogue that fans the output projection across 4 psum banks.

Host folds bv through Wo into the final bias (exact), sums the 8 bf16
partials in fp32, and adds bo.
"""

import sys

sys.path.insert(0, "/opt/trn_rl_repo")

import functools
import numpy as np
import ml_dtypes

D = 1024
H = 16
HD = 64
NCORES = 8
HPC = H // NCORES  # 2 heads per core
P = 128
CH = 512
S = 4096
NCHUNK = S // CH  # 8
ND = D // P  # 8
NT = S // P  # 32

USE_FP8 = True  # fp8e4 DoubleRow for the QK^T matmul

import os

TUNE_OP_TT = int(os.environ.get("TUNE_OP_TT", "6"))
TUNE_PIECE_TT = int(os.environ.get("TUNE_PIECE_TT", "1"))
TUNE_XT_BUFS = int(os.environ.get("TUNE_XT_BUFS", "6"))
TUNE_LAG = int(os.environ.get("TUNE_LAG", "2"))
TUNE_MK = int(os.environ.get("TUNE_MK", "3"))
# DVE-exp offload: every TUNE_DVE_M-th below-diag tile (0=off) for
# chunks >= TUNE_DVE_JMIN, skipping the last TUNE_DVE_TAIL tiles.
TUNE_DVE_M = int(os.environ.get("TUNE_DVE_M", "0"))
TUNE_DVE_JMIN = int(os.environ.get("TUNE_DVE_JMIN", "2"))
TUNE_DVE_TAIL = int(os.environ.get("TUNE_DVE_TAIL", "6"))
# columns given to DVE per offloaded tile (0 < soff < CH splits the tile
# between DVE [0:soff] and ACT [soff:]; soff=CH gives DVE the whole tile)
TUNE_DVE_SOFF = int(os.environ.get("TUNE_DVE_SOFF", "512"))
# per-chunk offload stride: "j6:2,j7:2" means every 2nd below-diag tile in
# chunks 6 and 7 (overrides TUNE_DVE_M/JMIN when set)
TUNE_DVE_MJ = os.environ.get("TUNE_DVE_MJ", "j6:2,j7:2")
# how many fifo positions a DVE tile's AV may be deferred (soak time)
TUNE_DVE_ROT = int(os.environ.get("TUNE_DVE_ROT", "3"))
TUNE_WARM = int(os.environ.get("TUNE_WARM", "8"))
TUNE_LATE = int(os.environ.get("TUNE_LATE", "0"))
# fp8e4m3 X and Wq/Wk (DoubleRow) for the Q,K projections; V stays bf16
TUNE_F8QK = int(os.environ.get("TUNE_F8QK", "0"))
F8_SCALE = 32.0  # host multiplies Wq/Wk by this; folded back at the convert


def register_exp_ops():
    """Two chained custom-DVE ops computing exp(x/8) as
    (1+u+u^2/2+u^3/6)^128 with u = x/(8*128): op1 = deg-3 Horner then one
    square (8 ALU stages), op2 = six squarings. f32 throughout; max rel
    err ~1.4e-5 over |x/8|<=4 (HW-validated)."""
    import concourse.dve_ops as dvo
    from concourse.dve_ops import DveOp
    from concourse.dve_spec import Spec, Src0, C0, C1, C2, One, lower
    from concourse.dve_uop import DveOpSpec

    if "ATT_EXP_BASE" in dvo._SUB_OPCODE_FOR_NAME:
        by_name = {op.name: op for op in dvo.OPS}
        return by_name["ATT_EXP_BASE"], by_name["ATT_EXP_SQ6"]

    u = Src0 * C0
    a1 = u * C1 + C2
    a2 = a1 * u + One
    a3 = a2 * u + One
    body1 = a3 * a3

    def ref1(in0, in1, s0, s1, imm2):
        x = in0.astype(np.float32)
        uu = x * np.float32(s0)
        b = ((uu * np.float32(s1) + np.float32(imm2)) * uu + 1.0) * uu + 1.0
        b = b.astype(np.float32)
        return (b * b).astype(np.float32)

    s = Src0
    for _ in range(6):
        s = s * s
    body2 = s

    def ref2(in0, in1, s0, s1, imm2):
        y = in0.astype(np.float32)
        for _ in range(6):
            y = (y * y).astype(np.float32)
        return y

    ops = []
    for name, body, ref in (
        ("ATT_EXP_BASE", body1, ref1),
        ("ATT_EXP_SQ6", body2, ref2),
    ):
        row = max(dvo._SUB_OPCODE_FOR_NAME.values()) + 1
        assert row < 0x20, row
        dvo._SUB_OPCODE_FOR_NAME[name] = row
        spec = Spec(body=body, reference=ref)
        shas = {}
        for ver in ("v3", "v4"):
            uops = lower(spec, ver=ver)
            s_ = DveOpSpec(name=name, opcode=row, uops=uops, rd1_en=False)
            shas[ver] = s_.sha(ver)
        op = DveOp(name, spec, subdim=False, uops_sha=shas)
        dvo.OPS.append(op)
        ops.append(op)
    return ops[0], ops[1]

BF16 = ml_dtypes.bfloat16

# emission-order labels per (engine, opcode) for profiling
LABELS = {}


def _lab(key, s):
    LABELS.setdefault(key, []).append(s)


def build_nc():
    import concourse.bacc as bacc
    import concourse.mybir as mybir
    from concourse import tile

    exp_base_op, exp_sq6_op = register_exp_ops()

    f32 = mybir.dt.float32
    bf16 = mybir.dt.bfloat16
    f8 = mybir.dt.float8e4
    ADD = mybir.AluOpType.add
    MULT = mybir.AluOpType.mult
    EXP = mybir.ActivationFunctionType.Exp
    DR = mybir.MatmulPerfMode.DoubleRow

    nc = bacc.Bacc("TRN2", target_bir_lowering=False, debug=False)

    xt_d = nc.dram_tensor("xt", [D, S], bf16, kind="ExternalInput")
    wqk0_d = nc.dram_tensor("wqk0", [D, 2 * P], bf16, kind="ExternalInput")
    if TUNE_F8QK:
        # DoubleRow-packed fp8 X and Wqk on partitions 0-63 only (DR from
        # base partition 64 faults on HW): [q, dt, i, ...] holds row
        # d = dt*128 + i*64 + q (host-prepared)
        x8_d = nc.dram_tensor(
            "x8", [64, NCHUNK, ND, 2, CH], f8, kind="ExternalInput"
        )
        w8_d = nc.dram_tensor(
            "w8", [64, ND, 2, 2 * P], f8, kind="ExternalInput"
        )
    bqk0_d = nc.dram_tensor("bqk0", [2 * P], f32, kind="ExternalInput")
    wv_d = nc.dram_tensor("wv", [P, ND * P], bf16, kind="ExternalInput")
    wo_d = nc.dram_tensor("wo", [P, D], bf16, kind="ExternalInput")
    masks_d = nc.dram_tensor("masks", [P, 2 * P], bf16, kind="ExternalInput")
    ident_d = nc.dram_tensor("ident", [P, P], f32, kind="ExternalInput")
    yt_d = nc.dram_tensor("yt", [D, S], bf16, kind="ExternalOutput")

    with tile.TileContext(nc) as tc:
        with (
            tc.tile_pool(name="consts", bufs=1) as consts,
            tc.tile_pool(name="xtp", bufs=TUNE_XT_BUFS) as xtp,
            tc.tile_pool(name="f8p", bufs=4) as f8p,
            tc.tile_pool(name="ptp", bufs=36) as ptp,
            tc.tile_pool(name="otp", bufs=6) as otp,
            tc.tile_pool(name="otTp", bufs=2) as otTp,
            tc.tile_pool(name="rcp", bufs=4) as rcp,
            tc.tile_pool(name="ytp", bufs=2) as ytp,
            tc.tile_pool(name="dxp", bufs=3) as dxp,
            tc.tile_pool(name="x8p", bufs=4) as x8p,
            tc.tile_pool(name="scp", bufs=2, space="PSUM") as scp,
            tc.tile_pool(name="avp", bufs=1, space="PSUM") as avp,
            tc.tile_pool(name="qkvp", bufs=1, space="PSUM") as qkvp,
            tc.tile_pool(name="ytpp", bufs=1, space="PSUM") as ytpp,
        ):
            # ---- persistent SBUF ----
            wv_sb = consts.tile([P, ND, P], bf16)
            wo_sb = consts.tile([P, ND, P], bf16)
            if TUNE_F8QK:
                w8_sb = consts.tile([64, ND, 2, 2 * P], f8)
            else:
                wqk0_sb = consts.tile([P, ND, 2 * P], bf16)
            bqk0_sb = consts.tile([P, 2], f32)
            masks_sb = consts.tile([P, 2, P], bf16)
            maskA_sb = masks_sb[:, 0]
            maskB_sb = masks_sb[:, 1]
            ident_sb = consts.tile([P, P], f32)
            # per-chunk tensors (separate tiles -> precise deps)
            v_sbs = [
                consts.tile([P, 4, HPC, 65], bf16, name=f"v{c}")
                for c in range(NCHUNK)
            ]
            if USE_FP8:
                qt_drs = [
                    consts.tile([64, 2, CH], f8, name=f"qdr{c}")
                    for c in range(NCHUNK)
                ]
                kt_drs = [
                    consts.tile([64, 2, CH], f8, name=f"kdr{c}")
                    for c in range(NCHUNK)
                ]
                # chunks 0-1 also get bf16 Q/K so the early scores skip
                # the fp8 rearrange round-trip (startup critical path)
                qt_sbs = [
                    consts.tile([P, CH], bf16, name=f"q{c}") for c in range(3)
                ]
                kt_sbs = [
                    consts.tile([P, CH], bf16, name=f"k{c}") for c in range(3)
                ]
            else:
                qt_sbs = [
                    consts.tile([P, CH], bf16, name=f"q{c}")
                    for c in range(NCHUNK)
                ]
                kt_sbs = [
                    consts.tile([P, CH], bf16, name=f"k{c}")
                    for c in range(NCHUNK)
                ]

            b_first_sb = bqk0_sb
            b_first_d = bqk0_d
            warm = consts.tile([P, CH], bf16)
            nc.gpsimd.memset(warm[:], 0.0)
            for c in range(NCHUNK):
                nc.gpsimd.memset(v_sbs[c][:, :, :, 64:65], 1.0)

            late_dr = []  # deferred fp8 rearrange DMAs for chunks 0-1

            # AV accumulators: [s, head, stile pair, 0:65 used]
            # two 1-bank tiles so early stile divs don't serialize later AVs
            avs = [
                avp.tile([P, HPC, 2, P], f32, name=f"av{b}") for b in range(2)
            ]

            def make_qkv_pieces(c, between=None):
                """QKV for chunk c as a list of emit-closures.

                X^T arrives in two half-tiles so the first Q matmuls can
                start as soon as the first half lands. `between` emits
                extra DMAs between the halves (startup weight staging).
                """
                xt_h = [
                    xtp.tile([P, ND // 2, CH], bf16, tag="xt", name="xt")
                    for _ in range(2)
                ]
                x8 = None
                if TUNE_F8QK:
                    x8 = x8p.tile([64, ND, 2, CH], f8, tag="x8", name="x8")
                    for g in range(2):
                        nc.sync.dma_start(
                            x8[:, 4 * g : 4 * g + 4],
                            x8_d[:, c, 4 * g : 4 * g + 4],
                        )
                        if g == 0 and between is not None:
                            between()
                for g in range(2):
                    nc.sync.dma_start(
                        xt_h[g][:],
                        xt_d[
                            g * (D // 2) : (g + 1) * (D // 2),
                            c * CH : (c + 1) * CH,
                        ].rearrange("(dt p) s -> p dt s", p=P),
                    )
                    if g == 0 and between is not None and not TUNE_F8QK:
                        between()

                def xt_t(d):
                    return xt_h[d // 4][:, d % 4]

                def qk_piece(t):  # 0 = Q, 1 = K
                    cell = [None]

                    def f(drange=(0, ND), tail=True):
                        b_sb = bqk0_sb
                        pool = ytpp if (c == 0 and t == 1) else qkvp
                        tag = "ytps" if (c == 0 and t == 1) else "qv"
                        if cell[0] is None:
                            cell[0] = pool.tile(
                                [P, CH], f32, tag=tag, name=f"qk{t}"
                            )
                        ps = cell[0]
                        for d in range(*drange):
                            _lab("mm", f"qk{t}.c{c}.d{d}")
                            if TUNE_F8QK:
                                nc.tensor.matmul(
                                    ps[:],
                                    w8_sb[:, d, :, t * P : (t + 1) * P],
                                    x8[:, d],
                                    start=(d == 0),
                                    stop=(d == ND - 1),
                                    perf_mode=DR,
                                )
                            else:
                                nc.tensor.matmul(
                                    ps[:],
                                    wqk0_sb[:, d, t * P : (t + 1) * P],
                                    xt_t(d)[:, :],
                                    start=(d == 0),
                                    stop=(d == ND - 1),
                                )
                        if not tail:
                            return

                        def cvt(out_ap):
                            if TUNE_F8QK:
                                nc.vector.tensor_scalar(
                                    out=out_ap,
                                    in0=ps[:],
                                    scalar1=1.0 / F8_SCALE,
                                    scalar2=b_sb[:, t : t + 1],
                                    op0=MULT,
                                    op1=ADD,
                                )
                            else:
                                nc.vector.tensor_scalar(
                                    out=out_ap,
                                    in0=ps[:],
                                    scalar1=b_sb[:, t : t + 1],
                                    scalar2=None,
                                    op0=ADD,
                                )

                        if USE_FP8:
                            if c <= 1:
                                cvt([qt_sbs, kt_sbs][t][c][:])
                                if t == 0:
                                    # qt_drs[0..1] are never read: chunks
                                    # 0-1 run their scores in bf16
                                    return
                            st = f8p.tile([P, CH], f8, tag="f8", name="f8st")
                            cvt(st[:])
                            dr_t = [qt_drs, kt_drs][t][c]

                            def emit_dr(dr_t=dr_t, st=st):
                                # natural [h0|h1] order -> dr layout
                                for h in range(HPC):
                                    for i in range(2):
                                        nc.sync.dma_start(
                                            dr_t[32 * h : 32 * h + 32, i, :],
                                            st[
                                                64 * h + 32 * i : 64 * h
                                                + 32 * i
                                                + 32,
                                                :,
                                            ],
                                        )

                            if c <= 1:
                                # keep these off the startup HWDGE queue:
                                # first consumers are chunks 1-2
                                late_dr.append(emit_dr)
                            else:
                                emit_dr()
                        else:
                            cvt([qt_sbs, kt_sbs][t][c][:])

                    return f

                def v_piece(i0):  # s-tiles i0, i0+1
                    def f():
                        pool = ytpp if (c == 0 and i0 == 2) else qkvp
                        tag = "ytps" if (c == 0 and i0 == 2) else "qv"
                        ps = pool.tile([P, CH], f32, tag=tag, name="vps")
                        psv = ps[:].rearrange("p (i f) -> p i f", f=P)
                        for i in (i0, i0 + 1):
                            for d in range(ND):
                                _lab("mm", f"v.c{c}.i{i}.d{d}")
                                nc.tensor.matmul(
                                    psv[:, i, :],
                                    xt_t(d)[:, i * P : (i + 1) * P],
                                    wv_sb[:, d, :],
                                    start=(i == i0 and d == 0),
                                    stop=(i == i0 + 1 and d == ND - 1),
                                )
                        nc.vector.tensor_copy(
                            v_sbs[c][:, i0 : i0 + 2, :, 0:64],
                            psv[:, i0 : i0 + 2, :].rearrange(
                                "p i (h f) -> p i h f", f=HD
                            ),
                        )

                    return f

                return [qk_piece(0), qk_piece(1), v_piece(0), v_piece(2)]

            def emit_scores(j, tt, off=False):
                """QK^T for t-tile tt against s-chunk j, with causal mask.

                off=True: below-diag tile whose first SOFF columns go to a
                1-bank PSUM tile borrowed from the outproj pool (DVE-exp'd
                off the main scores ping-pong); cols [SOFF:] go through scp
                as usual (ACT-exp'd).
                """
                k = tt - 4 * j
                o = P * k if k >= 0 else 0
                SOFF = 256
                sc = scp.tile([P, HPC, CH], f32, tag="sc", name="sc")
                s_off = None
                if off:
                    s_off = qkvp.tile(
                        [P, HPC, SOFF], f32, tag="qv", name="sOff"
                    )
                    assert USE_FP8 and j > 1 and k < 0
                    for h in range(HPC):
                        lhsT = kt_drs[tt // 4][
                            32 * h : 32 * h + 32,
                            :,
                            (tt % 4) * P : (tt % 4 + 1) * P,
                        ]
                        _lab("mm", f"scOff.j{j}.t{tt}.h{h}")
                        nc.tensor.matmul(
                            s_off[:, h, :],
                            lhsT,
                            qt_drs[j][32 * h : 32 * h + 32, :, 0:SOFF],
                            start=(h == 0),
                            stop=(h == HPC - 1),
                            perf_mode=DR,
                        )
                    o = SOFF
                for h in range(HPC):
                    if USE_FP8 and j <= 1:
                        # early chunks read the natural-order bf16 Q/K tiles
                        def mm(a, b, start, stop):
                            _lab("mm", f"sc.j{j}.t{tt}.h{h}")
                            nc.tensor.matmul(
                                sc[:, h, a:b],
                                kt_sbs[tt // 4][
                                    64 * h : 64 * h + 64,
                                    (tt % 4) * P : (tt % 4 + 1) * P,
                                ],
                                qt_sbs[j][64 * h : 64 * h + 64, a:b],
                                start=start,
                                stop=stop,
                            )

                    elif USE_FP8:
                        lhsT = kt_drs[tt // 4][
                            32 * h : 32 * h + 32, :, (tt % 4) * P : (tt % 4 + 1) * P
                        ]

                        def mm(a, b, start, stop):
                            _lab("mm", f"sc.j{j}.t{tt}.h{h}")
                            nc.tensor.matmul(
                                sc[:, h, a:b],
                                lhsT,
                                qt_drs[j][32 * h : 32 * h + 32, :, a:b],
                                start=start,
                                stop=stop,
                                perf_mode=DR,
                            )
                    else:
                        lhsT = kt_sbs[tt // 4][
                            64 * h : 64 * h + 64, (tt % 4) * P : (tt % 4 + 1) * P
                        ]

                        def mm(a, b, start, stop):
                            _lab("mm", f"sc.j{j}.t{tt}.h{h}")
                            nc.tensor.matmul(
                                sc[:, h, a:b],
                                lhsT,
                                qt_sbs[j][64 * h : 64 * h + 64, a:b],
                                start=start,
                                stop=stop,
                            )

                    if k < 0:
                        mm(SOFF if off else 0, CH, True, True)
                    else:
                        # one group per bank: the first pass zeroes the whole
                        # 2KB zero-region, later passes accumulate
                        if o + P < CH:
                            mm(o + P, CH, True, False)
                            mm(o, o + P, False, False)
                        else:
                            mm(o, o + P, True, False)
                        # diagonal 128-block [o:o+P] gets the ramp mask added
                        _lab("mm", f"mask.j{j}.t{tt}.h{h}")
                        nc.tensor.matmul(
                            sc[:, h, o : o + P],
                            maskA_sb[:, :],
                            maskB_sb[:, :],
                            start=False,
                            stop=True,
                        )
                return sc, o, s_off

            def emit_exp(j, tt, sc, o, s_off, late=False):
                pt = ptp.tile([P, HPC, CH], bf16, tag="pt", name="pt")
                if s_off is not None:
                    so = o  # = SOFF
                    tmp = dxp.tile([P, HPC, CH], f32, tag="dx", name="dx")
                    _lab("dve", f"expA.j{j}.t{tt}")
                    nc.vector._custom_dve(
                        exp_base_op,
                        out=tmp[:, :, 0:so],
                        in0=s_off[:],
                        s0=0.125 / 128.0,
                        s1=1.0 / 6.0,
                        imm2=0.5,
                    )
                    _lab("dve", f"expB.j{j}.t{tt}")
                    nc.vector._custom_dve(
                        exp_sq6_op, out=pt[:, :, 0:so], in0=tmp[:, :, 0:so]
                    )
                    nc.scalar.activation(
                        pt[:, :, o:], sc[:, :, o:], EXP, scale=0.125
                    )
                    return pt
                k = tt - 4 * j
                lo = max(o + P, CH - 256) if k >= 0 else CH - 256
                if late and lo < CH:
                    # kernel tail: no later scores wait on this scp buf, so
                    # DVE can exp the clean right columns straight from it
                    tmp = dxp.tile([P, HPC, CH], f32, tag="dx", name="dx")
                    _lab("dve", f"expL.j{j}.t{tt}")
                    nc.vector._custom_dve(
                        exp_base_op,
                        out=tmp[:, :, lo:],
                        in0=sc[:, :, lo:],
                        s0=0.125 / 128.0,
                        s1=1.0 / 6.0,
                        imm2=0.5,
                    )
                    nc.vector._custom_dve(
                        exp_sq6_op, out=pt[:, :, lo:], in0=tmp[:, :, lo:]
                    )
                    if o < lo:
                        nc.scalar.activation(
                            pt[:, :, o:lo], sc[:, :, o:lo], EXP, scale=0.125
                        )
                    return pt
                nc.scalar.activation(
                    pt[:, :, o:], sc[:, :, o:], EXP, scale=0.125
                )
                return pt

            def emit_av(j, tt, pt):
                k = tt - 4 * j
                vt = v_sbs[tt // 4]
                for i in range(max(k, 0), 4):
                    for h in range(HPC):
                        _lab("mm", f"av.j{j}.t{tt}.i{i}.h{h}")
                        nc.tensor.matmul(
                            avs[i // 2][:, h, i % 2, 0:65],
                            pt[:, h, i * P : (i + 1) * P],
                            vt[:, tt % 4, h, :],
                            # one accumulation group per bank (zero region):
                            # first pass zeroes it, last pass closes it
                            start=(tt == 0 and i % 2 == 0 and h == 0),
                            stop=(tt == 4 * j + i and i % 2 == 1 and h == 1),
                        )

            def emit_div(j, i, otT, tail=False):
                av = avs[i // 2]
                rc = rcp.tile([P, HPC, 1], f32, tag="rc", name="rc")
                nc.vector.reciprocal(rc[:], av[:, :, i % 2, 64:65])
                ot = otp.tile([P, P], f32, tag="ot", name="ot")
                for h in range(HPC):
                    if False:
                        nc.scalar.mul(
                            ot[:, HD * h : HD * (h + 1)],
                            av[:, h, i % 2, 0:64],
                            rc[:, h, :],
                        )
                    else:
                        nc.vector.tensor_scalar(
                            out=ot[:, HD * h : HD * (h + 1)],
                            in0=av[:, h, i % 2, 0:64],
                            scalar1=rc[:, h, :],
                            scalar2=None,
                            op0=MULT,
                        )
                # PE-transpose (f32) into av's dead region, then copy out
                # to SBUF as bf16; avoids the serialized HWDGE path
                tp = av[:, 0, i % 2, 0:P]
                _lab("mm", f"tp.j{j}.i{i}")
                nc.tensor.transpose(tp, ot[:], ident_sb[:])
                nc.vector.tensor_copy(otT[:, i, :], tp)

            def make_outproj_pieces(j, otT):
                yt_sb = ytp.tile([P, ND, CH], bf16, tag="yt", name="yt")
                rhs = otT[:].rearrange("p i f -> p (i f)")

                def piece(e):
                    def f():
                        ps = ytpp.tile([P, CH], f32, tag="ytps", name="ytps")
                        _lab("mm", f"op.j{j}.e{e}")
                        nc.tensor.matmul(
                            ps[:], wo_sb[:, e, :], rhs, start=True, stop=True
                        )
                        nc.vector.tensor_copy(yt_sb[:, e, :], ps[:])
                        if e == ND - 1:
                            nc.sync.dma_start(
                                yt_d[:, j * CH : (j + 1) * CH].rearrange(
                                    "(e p) s -> p e s", p=P
                                ),
                                yt_sb[:],
                            )

                    return f

                return [piece(e) for e in range(ND)]

            op_q = []
            otTs = {}

            def emit_div_pair(jj, pair, otT, tail=False):
                # one reciprocal for both stiles; transposes land in the
                # dead av region and copy out to SBUF in one merged move
                av = avs[pair]
                rc = rcp.tile([P, HPC, 2, 1], f32, tag="rc", name="rc")
                nc.vector.reciprocal(rc[:], av[:, :, 0:2, 64:65])
                for i in (2 * pair, 2 * pair + 1):
                    ot = otp.tile([P, P], f32, tag="ot", name="ot")
                    for h in range(HPC):
                        nc.vector.tensor_scalar(
                            out=ot[:, HD * h : HD * (h + 1)],
                            in0=av[:, h, i % 2, 0:64],
                            scalar1=rc[:, h, i % 2],
                            scalar2=None,
                            op0=MULT,
                        )
                    tp = av[:, 0, i % 2, 0:P]
                    _lab("mm", f"tp.j{jj}.i{i}")
                    nc.tensor.transpose(tp, ot[:], ident_sb[:])
                nc.vector.tensor_copy(
                    otT[:, 2 * pair : 2 * pair + 2, :], av[:, 0, 0:2, 0:P]
                )

            tail_state = {}

            def emit_tail_half(jj, half, otT):
                a, b = half * 256, half * 256 + 256
                if half == 0:
                    tail_state["yt"] = ytp.tile(
                        [P, ND, CH], bf16, tag="yt", name="yt"
                    )
                yt_sb = tail_state["yt"]
                rhs = otT[:, 2 * half : 2 * half + 2, :].rearrange(
                    "p i f -> p (i f)"
                )
                for e in range(ND):
                    if e % 4 == 0:
                        ps = ytpp.tile([P, CH], f32, tag="ytps", name="ytt")
                        ps = ps[:, 0:256]
                    elif e % 4 == 1:
                        ps = qkvp.tile([P, CH], f32, tag="qv", name="ytt")
                        ps = ps[:, 0:256]
                    else:
                        # scores pool banks are free at the tail
                        ps = scp.tile([P, HPC, CH], f32, tag="sc", name="ytt")
                        ps = ps[:, e % 2, 0:256]
                    _lab("mm", f"op.j{jj}.e{e}.h{half}")
                    nc.tensor.matmul(
                        ps, wo_sb[:, e, :], rhs, start=True, stop=True
                    )
                    if e % 2 == 0:
                        nc.scalar.copy(yt_sb[:, e, a:b], ps)
                    else:
                        nc.vector.tensor_copy(yt_sb[:, e, a:b], ps)
                    if e % 4 == 3:
                        nc.sync.dma_start(
                            yt_d[
                                (e - 3) * P : (e + 1) * P,
                                jj * CH + a : jj * CH + b,
                            ].rearrange("(e p) s -> p e s", p=P),
                            yt_sb[:, e - 3 : e + 1, a:b],
                        )

            def handle_pop(j):
                # let a DVE-exp'd tile soak: defer its AV behind the next
                # below-diag tile of the same chunk (start flag rides tt=0,
                # stop flags ride diag tiles — both excluded)
                if (
                    fifo[0][3] > 0
                    and fifo[0][1] != 0
                    and len(fifo) > 1
                    and fifo[1][0] == fifo[0][0]
                    and fifo[1][1] - 4 * fifo[1][0] < 0
                ):
                    e = fifo.pop(0)
                    e[3] -= 1
                    fifo.insert(1, e)
                jj, t0, p0_, _ = fifo.pop(0)
                emit_av(jj, t0, p0_)
                k0 = t0 - 4 * jj
                tail = jj == NCHUNK - 1
                if k0 == 1:
                    otTs[jj] = otTp.tile([P, 4, P], bf16, tag="otT", name="otT")
                    emit_div_pair(jj, 0, otTs[jj], tail=tail)
                elif k0 == 3:
                    emit_div_pair(jj, 1, otTs[jj], tail=tail)
                    if tail:
                        emit_tail_half(jj, 0, otTs[jj])
                        emit_tail_half(jj, 1, otTs.pop(jj))
                    else:
                        op_q.extend(make_outproj_pieces(jj, otTs.pop(jj)))

            # ---- merged software-pipelined stream ----
            wps = qkvp.tile([P, CH], f32, tag="qv", name="warmps")
            for w in range(TUNE_WARM):
                _lab("mm", f"warm{w}")
                nc.tensor.matmul(
                    wps[:],
                    warm[:, 0:P],
                    warm[:],
                    start=(w == 0),
                    stop=(w == TUNE_WARM - 1),
                )

            def stage_wqk_first_half():
                # wqk rows for d-tiles 0-3: all the first Q/K half needs
                if TUNE_F8QK:
                    nc.sync.dma_start(w8_sb[:, 0:4], w8_d[:, 0:4])
                else:
                    nc.sync.dma_start(
                        wqk0_sb[:, 0:4],
                        wqk0_d[0 : D // 2].rearrange(
                            "(dt p) f -> p dt f", p=P
                        ),
                    )

            p0 = make_qkv_pieces(0, between=stage_wqk_first_half)
            if TUNE_F8QK:
                nc.sync.dma_start(w8_sb[:, 4:8], w8_d[:, 4:8])
            else:
                nc.sync.dma_start(
                    wqk0_sb[:, 4:8],
                    wqk0_d[D // 2 : D].rearrange("(dt p) f -> p dt f", p=P),
                )
            nc.sync.dma_start(
                b_first_sb[:], b_first_d[:].rearrange("(c p) -> p c", p=P)
            )
            nc.sync.dma_start(
                wv_sb[:], wv_d[:].rearrange("p (dt f) -> p dt f", f=P)
            )
            qf, kf = p0[0], p0[1]
            qf(drange=(0, ND // 2), tail=False)
            kf(drange=(0, ND // 2), tail=False)
            qf(drange=(ND // 2, ND))
            kf(drange=(ND // 2, ND))
            # masks needed by the first diagonal exp (~10us in)
            nc.sync.dma_start(
                masks_sb[:], masks_d[:].rearrange("p (m f) -> p m f", f=P)
            )
            nc.sync.dma_start(ident_sb[:], ident_d[:])
            # xt(1) prefetch must beat the cold-path weight loads below
            p1 = make_qkv_pieces(1)
            # order: V01(0), Q(1), V23(0), K(1), V01(1), V23(1)
            pieces = {1: [p0[2], p1[0], p0[3], p1[1], p1[2], p1[3]]}
            nc.sync.dma_start(
                wo_sb[:], wo_d[:].rearrange("p (e f) -> p e f", f=P)
            )
            fifo = []
            dve_exp_ctr = [0]
            mj = {}
            if TUNE_DVE_MJ:
                for part in TUNE_DVE_MJ.split(","):
                    kj, vj = part.split(":")
                    mj[int(kj.lstrip("j"))] = int(vj)
            for j in range(NCHUNK):
                ntt = 4 * j + 4
                cur = pieces.pop(j + 1, [])
                for tt in range(ntt):
                    k = tt - 4 * j
                    lt = (ntt - tt) if j == NCHUNK - 1 else 999
                    m_here = mj.get(j, 0) if mj else (
                        TUNE_DVE_M
                        if TUNE_DVE_M > 0 and j > max(2, TUNE_DVE_JMIN - 1)
                        else 0
                    )
                    off = (
                        m_here > 0
                        and k < 0  # below-diag: no -1e30 masked entries
                        and USE_FP8
                        and lt > TUNE_DVE_TAIL
                    )
                    if off:
                        dve_exp_ctr[0] += 1
                        off = dve_exp_ctr[0] % m_here == 0
                    late = (
                        TUNE_LATE > 0
                        and j == NCHUNK - 1
                        and lt <= TUNE_DVE_TAIL
                    )
                    sc, o, s_off = emit_scores(j, tt, off=off)
                    pt = emit_exp(j, tt, sc, o, s_off, late=late)
                    fifo.append([j, tt, pt, TUNE_DVE_ROT if off else 0])
                    if tt >= TUNE_PIECE_TT and cur:
                        cur.pop(0)()
                        if j == 0 and cur:
                            cur.pop(0)()
                    if j in (1, 2) and late_dr:
                        late_dr.pop(0)()
                    if len(fifo) > (4 if j == 0 else TUNE_LAG):
                        handle_pop(j)
                    if tt == max(TUNE_MK, ntt // 2) and j + 2 < NCHUNK:
                        pieces[j + 2] = make_qkv_pieces(j + 2)
                    if tt >= TUNE_OP_TT and op_q:
                        op_q.pop(0)()
                if cur and j + 2 < NCHUNK:
                    # carry leftovers into the next chunk's piece stream so
                    # they don't lump up right before its first scores
                    pieces[j + 2] = cur + pieces.get(j + 2, [])
                else:
                    while cur:
                        cur.pop(0)()
            while fifo:
                handle_pop(NCHUNK - 1)
            while op_q:
                op_q.pop(0)()

    return nc


@functools.lru_cache(maxsize=1)
def _get_nc(S_arg=S):
    nc = build_nc()
    nc.compile()
    return nc


def _dr_pack(a):
    """[D, F] -> DoubleRow-packed [64, ND, 2, F]: row d = dt*128 + i*64 + q
    sits at [q, dt, i, :] (partitions 0-63 only)."""
    F = a.shape[1]
    return np.ascontiguousarray(
        a.reshape(ND, 2, 64, F).transpose(2, 0, 1, 3)
    )


def make_in_maps(input, Wqkv, bqkv, Wo):
    F8 = ml_dtypes.float8_e4m3fn
    x = np.asarray(input, dtype=np.float32).reshape(S, D)
    xt = np.ascontiguousarray(x.T).astype(BF16)
    Wqkv = np.asarray(Wqkv, dtype=np.float32)
    bqkv = np.asarray(bqkv, dtype=np.float32)
    Wo = np.asarray(Wo, dtype=np.float32)
    Wq, Wk, Wv = Wqkv[:, 0:D], Wqkv[:, D : 2 * D], Wqkv[:, 2 * D : 3 * D]
    bq, bk = bqkv[0:D], bqkv[D : 2 * D]
    if TUNE_F8QK:
        # x8: [64, NCHUNK, ND, 2, CH] chunk-major DR packing of X^T fp8
        x8_full = _dr_pack(np.ascontiguousarray(x.T).astype(F8))  # [64,8,2,S]
        x8 = np.ascontiguousarray(
            x8_full.reshape(64, ND, 2, NCHUNK, CH).transpose(0, 3, 1, 2, 4)
        )

    r = np.arange(P)
    maskA = np.where(r[:, None] < r[None, :], np.float32(-1e30), np.float32(0))
    maskB = (r[:, None] >= r[None, :]).astype(np.float32)
    maskA = np.ascontiguousarray(maskA.astype(BF16))
    maskB = np.ascontiguousarray(maskB.astype(BF16))
    ident = np.ascontiguousarray(np.eye(P, dtype=np.float32))

    in_maps = []
    for c in range(NCORES):
        hs = [HPC * c + i for i in range(HPC)]

        def headcols(W):
            return np.concatenate([W[:, h * HD : (h + 1) * HD] for h in hs], 1)

        def headvec(b):
            return np.concatenate([b[h * HD : (h + 1) * HD] for h in hs], 0)

        wq0, wk0 = headcols(Wq), headcols(Wk)
        bq0, bk0 = headvec(bq), headvec(bk)
        extra = {}
        if TUNE_F8QK:
            w8 = np.concatenate([wq0, wk0], axis=1) * np.float32(F8_SCALE)
            extra["x8"] = x8
            extra["w8"] = _dr_pack(w8.astype(F8))
        in_maps.append(
            {
                **extra,
                "xt": xt,
                "wqk0": np.ascontiguousarray(
                    np.concatenate([wq0, wk0], axis=1).astype(BF16)
                ),
                "bqk0": np.ascontiguousarray(
                    np.concatenate([bq0, bk0], axis=0).astype(np.float32)
                ),
                "wv": np.ascontiguousarray(
                    headcols(Wv)
                    .astype(BF16)
                    .reshape(ND, P, P)
                    .transpose(1, 0, 2)
                    .reshape(P, ND * P)
                ),
                "wo": np.ascontiguousarray(
                    Wo[hs[0] * HD : hs[0] * HD + HPC * HD, :].astype(BF16)
                ),
                "masks": np.ascontiguousarray(
                    np.concatenate([maskA, maskB], axis=1)
                ),
                "ident": ident,
            }
        )
    return in_maps


def kernel(input, Wqkv, bqkv, Wo, bo):
    from concourse.bass_utils import run_bass_kernel_spmd

    nc = _get_nc()
    in_maps = make_in_maps(input, Wqkv, bqkv, Wo)
    res = None
    last_exc = None
    for _attempt in range(3):  # transient NRT/device errors: retry
        try:
            res = run_bass_kernel_spmd(nc, in_maps, core_ids=list(range(NCORES)))
            break
        except Exception as e:  # noqa: BLE001
            last_exc = e
    if res is None:
        raise last_exc
    acc = np.zeros((D, S), np.float32)
    for r in res.results:
        acc += np.asarray(r["yt"], dtype=np.float32)
    y = np.ascontiguousarray(acc.T)
    bv = np.asarray(bqkv, np.float32)[2 * D : 3 * D]
    y += (bv @ np.asarray(Wo, np.float32) + np.asarray(bo, np.float32))[None, :]
    return y.reshape(1, S, D)



# revision 73
# speedup vs baseline: 1.0441x; 1.0441x over previous
"""Multi-head causal self-attention (B=1, S=4096, D=1024, H=16) on 8 TRN2
NeuronCores, tensor-parallel over heads (2 heads per core).

Engine plan (per core, modeled by TimelineSim):
  - PE: QKV projection bf16 (Q^T/K^T weight-stationary -> [feat, s]; V
    X-stationary -> [s, feat], so V needs no transpose), scores QK^T via
    fp8e4 DoubleRow (0.5 cyc/row; chunks 0-2 run bf16 to skip the fp8
    rearrange latency at startup), causal mask added in-PSUM by a ramp
    matmul (-1e30*max(t-s,0) = A^T B, so exp() of masked entries is 0),
    AV with P^T stationary and [V|1] moving (N=65/pass; the denominator
    rides along as column 64), ot [s,f] -> ot^T via PE transpose into a
    dead AV-psum region, output projection bf16.
  - ACT: the softmax exp only (scale=1/8 folded in), one inst per t-tile;
    at the tail it also takes half the output-projection copies.
  - DVE: all psum->sbuf copies (GPSIMD cannot touch PSUM), reciprocal +
    per-partition divide (the [s, f] layout makes the denominator a
    per-partition scalar).
  - SP: all DMAs. fp8 Q/K reach the DoubleRow [32, 2, *] layout via 4
    sbuf->sbuf transfers per tensor, deferred off the startup queue.

PSUM (8 banks, one accumulation group per 2KB zero-region at a time):
  scores pool 2x[128, 2, 512] (4) | AV 2x[128, 2, 2, 128] (2) | QKV (1) |
  outproj (1).

Schedule: one merged software-pipelined stream over (chunk j, t-tile tt):
scores -> exp -> (lag 2) AV, with QKV(j+1) pieces and outproj(j-1) pieces
interleaved between t-tiles, divisions per stile-pair as their AV closes,
PE warmup matmuls during the initial DMAs (pstate ramp), and a split
last-chunk epilogue that fans the output projection across 4 psum banks.

Host folds bv through Wo into the final bias (exact), sums the 8 bf16
partials in fp32, and adds bo.
"""

import sys

sys.path.insert(0, "/opt/trn_rl_repo")

import functools
import numpy as np
import ml_dtypes

D = 1024
H = 16
HD = 64
NCORES = 8
HPC = H // NCORES  # 2 heads per core
P = 128
CH = 512
S = 4096
NCHUNK = S // CH  # 8
ND = D // P  # 8
NT = S // P  # 32

USE_FP8 = True  # fp8e4 DoubleRow for the QK^T matmul

import os

TUNE_OP_TT = int(os.environ.get("TUNE_OP_TT", "6"))
TUNE_PIECE_TT = int(os.environ.get("TUNE_PIECE_TT", "1"))
TUNE_XT_BUFS = int(os.environ.get("TUNE_XT_BUFS", "6"))
TUNE_LAG = int(os.environ.get("TUNE_LAG", "2"))
TUNE_MK = int(os.environ.get("TUNE_MK", "3"))
# DVE-exp offload: every TUNE_DVE_M-th below-diag tile (0=off) for
# chunks >= TUNE_DVE_JMIN, skipping the last TUNE_DVE_TAIL tiles.
TUNE_DVE_M = int(os.environ.get("TUNE_DVE_M", "0"))
TUNE_DVE_JMIN = int(os.environ.get("TUNE_DVE_JMIN", "2"))
TUNE_DVE_TAIL = int(os.environ.get("TUNE_DVE_TAIL", "6"))
# columns given to DVE per offloaded tile (0 < soff < CH splits the tile
# between DVE [0:soff] and ACT [soff:]; soff=CH gives DVE the whole tile)
TUNE_DVE_SOFF = int(os.environ.get("TUNE_DVE_SOFF", "512"))
# per-chunk offload stride: "j6:2,j7:2" means every 2nd below-diag tile in
# chunks 6 and 7 (overrides TUNE_DVE_M/JMIN when set)
TUNE_DVE_MJ = os.environ.get("TUNE_DVE_MJ", "j6:2,j7:2")
# how many fifo positions a DVE tile's AV may be deferred (soak time)
TUNE_DVE_ROT = int(os.environ.get("TUNE_DVE_ROT", "3"))
TUNE_WARM = int(os.environ.get("TUNE_WARM", "8"))
TUNE_LATE = int(os.environ.get("TUNE_LATE", "0"))
TUNE_DVE_FAKE = int(os.environ.get("TUNE_DVE_FAKE", "0"))
TUNE_DVE_ACTALL = int(os.environ.get("TUNE_DVE_ACTALL", "0"))
TUNE_DVE_PROBE = int(os.environ.get("TUNE_DVE_PROBE", "0"))
# fp8e4m3 X and Wq/Wk (DoubleRow) for the Q,K projections; V stays bf16
TUNE_F8QK = int(os.environ.get("TUNE_F8QK", "0"))
F8_SCALE = 32.0  # host multiplies Wq/Wk by this; folded back at the convert


def register_exp_ops():
    """Two chained custom-DVE ops computing exp(x/8) as
    (1+u+u^2/2+u^3/6)^128 with u = x/(8*128): op1 = deg-3 Horner then one
    square (8 ALU stages), op2 = six squarings. f32 throughout; max rel
    err ~1.4e-5 over |x/8|<=4 (HW-validated)."""
    import concourse.dve_ops as dvo
    from concourse.dve_ops import DveOp
    from concourse.dve_spec import Spec, Src0, C0, C1, C2, One, lower
    from concourse.dve_uop import DveOpSpec

    if "ATT_EXP_BASE" in dvo._SUB_OPCODE_FOR_NAME:
        by_name = {op.name: op for op in dvo.OPS}
        return by_name["ATT_EXP_BASE"], by_name["ATT_EXP_SQ6"]

    u = Src0 * C0
    a1 = u * C1 + C2
    a2 = a1 * u + One
    a3 = a2 * u + One
    body1 = a3 * a3

    def ref1(in0, in1, s0, s1, imm2):
        x = in0.astype(np.float32)
        uu = x * np.float32(s0)
        b = ((uu * np.float32(s1) + np.float32(imm2)) * uu + 1.0) * uu + 1.0
        b = b.astype(np.float32)
        return (b * b).astype(np.float32)

    s = Src0
    for _ in range(6):
        s = s * s
    body2 = s

    def ref2(in0, in1, s0, s1, imm2):
        y = in0.astype(np.float32)
        for _ in range(6):
            y = (y * y).astype(np.float32)
        return y

    ops = []
    for name, body, ref in (
        ("ATT_EXP_BASE", body1, ref1),
        ("ATT_EXP_SQ6", body2, ref2),
    ):
        row = max(dvo._SUB_OPCODE_FOR_NAME.values()) + 1
        assert row < 0x20, row
        dvo._SUB_OPCODE_FOR_NAME[name] = row
        spec = Spec(body=body, reference=ref)
        shas = {}
        for ver in ("v3", "v4"):
            uops = lower(spec, ver=ver)
            s_ = DveOpSpec(name=name, opcode=row, uops=uops, rd1_en=False)
            shas[ver] = s_.sha(ver)
        op = DveOp(name, spec, subdim=False, uops_sha=shas)
        dvo.OPS.append(op)
        ops.append(op)
    return ops[0], ops[1]

BF16 = ml_dtypes.bfloat16

# emission-order labels per (engine, opcode) for profiling
LABELS = {}


def _lab(key, s):
    LABELS.setdefault(key, []).append(s)


def build_nc():
    import concourse.bacc as bacc
    import concourse.mybir as mybir
    from concourse import tile

    exp_base_op, exp_sq6_op = register_exp_ops()

    f32 = mybir.dt.float32
    bf16 = mybir.dt.bfloat16
    f8 = mybir.dt.float8e4
    ADD = mybir.AluOpType.add
    MULT = mybir.AluOpType.mult
    EXP = mybir.ActivationFunctionType.Exp
    DR = mybir.MatmulPerfMode.DoubleRow

    nc = bacc.Bacc("TRN2", target_bir_lowering=False, debug=False)

    xt_d = nc.dram_tensor("xt", [D, S], bf16, kind="ExternalInput")
    wqk0_d = nc.dram_tensor("wqk0", [D, 2 * P], bf16, kind="ExternalInput")
    if TUNE_F8QK:
        # DoubleRow-packed fp8 X and Wqk on partitions 0-63 only (DR from
        # base partition 64 faults on HW): [q, dt, i, ...] holds row
        # d = dt*128 + i*64 + q (host-prepared)
        x8_d = nc.dram_tensor(
            "x8", [64, NCHUNK, ND, 2, CH], f8, kind="ExternalInput"
        )
        w8_d = nc.dram_tensor(
            "w8", [64, ND, 2, 2 * P], f8, kind="ExternalInput"
        )
    bqk0_d = nc.dram_tensor("bqk0", [2 * P], f32, kind="ExternalInput")
    wv_d = nc.dram_tensor("wv", [P, ND * P], bf16, kind="ExternalInput")
    wo_d = nc.dram_tensor("wo", [P, D], bf16, kind="ExternalInput")
    masks_d = nc.dram_tensor("masks", [P, 2 * P], bf16, kind="ExternalInput")
    ident_d = nc.dram_tensor("ident", [P, P], f32, kind="ExternalInput")
    yt_d = nc.dram_tensor("yt", [D, S], bf16, kind="ExternalOutput")

    with tile.TileContext(nc) as tc:
        with (
            tc.tile_pool(name="consts", bufs=1) as consts,
            tc.tile_pool(name="xtp", bufs=TUNE_XT_BUFS) as xtp,
            tc.tile_pool(name="f8p", bufs=4) as f8p,
            tc.tile_pool(name="ptp", bufs=36) as ptp,
            tc.tile_pool(name="otp", bufs=6) as otp,
            tc.tile_pool(name="otTp", bufs=2) as otTp,
            tc.tile_pool(name="rcp", bufs=4) as rcp,
            tc.tile_pool(name="ytp", bufs=2) as ytp,
            tc.tile_pool(name="dxp", bufs=3) as dxp,
            tc.tile_pool(name="x8p", bufs=4) as x8p,
            tc.tile_pool(name="scp", bufs=2, space="PSUM") as scp,
            tc.tile_pool(name="avp", bufs=1, space="PSUM") as avp,
            tc.tile_pool(name="qkvp", bufs=1, space="PSUM") as qkvp,
            tc.tile_pool(name="ytpp", bufs=1, space="PSUM") as ytpp,
        ):
            # ---- persistent SBUF ----
            wv_sb = consts.tile([P, ND, P], bf16)
            wo_sb = consts.tile([P, ND, P], bf16)
            if TUNE_F8QK:
                w8_sb = consts.tile([64, ND, 2, 2 * P], f8)
            else:
                wqk0_sb = consts.tile([P, ND, 2 * P], bf16)
            bqk0_sb = consts.tile([P, 2], f32)
            masks_sb = consts.tile([P, 2, P], bf16)
            maskA_sb = masks_sb[:, 0]
            maskB_sb = masks_sb[:, 1]
            ident_sb = consts.tile([P, P], f32)
            # per-chunk tensors (separate tiles -> precise deps)
            v_sbs = [
                consts.tile([P, 4, HPC, 65], bf16, name=f"v{c}")
                for c in range(NCHUNK)
            ]
            if USE_FP8:
                qt_drs = [
                    consts.tile([64, 2, CH], f8, name=f"qdr{c}")
                    for c in range(NCHUNK)
                ]
                kt_drs = [
                    consts.tile([64, 2, CH], f8, name=f"kdr{c}")
                    for c in range(NCHUNK)
                ]
                # chunks 0-1 also get bf16 Q/K so the early scores skip
                # the fp8 rearrange round-trip (startup critical path)
                qt_sbs = [
                    consts.tile([P, CH], bf16, name=f"q{c}") for c in range(3)
                ]
                kt_sbs = [
                    consts.tile([P, CH], bf16, name=f"k{c}") for c in range(3)
                ]
            else:
                qt_sbs = [
                    consts.tile([P, CH], bf16, name=f"q{c}")
                    for c in range(NCHUNK)
                ]
                kt_sbs = [
                    consts.tile([P, CH], bf16, name=f"k{c}")
                    for c in range(NCHUNK)
                ]

            b_first_sb = bqk0_sb
            b_first_d = bqk0_d
            warm = consts.tile([P, CH], bf16)
            nc.gpsimd.memset(warm[:], 0.0)
            for c in range(NCHUNK):
                nc.gpsimd.memset(v_sbs[c][:, :, :, 64:65], 1.0)

            late_dr = []  # deferred fp8 rearrange DMAs for chunks 0-1

            # AV accumulators: [s, head, stile pair, 0:65 used]
            # two 1-bank tiles so early stile divs don't serialize later AVs
            avs = [
                avp.tile([P, HPC, 2, P], f32, name=f"av{b}") for b in range(2)
            ]

            def make_qkv_pieces(c, between=None):
                """QKV for chunk c as a list of emit-closures.

                X^T arrives in two half-tiles so the first Q matmuls can
                start as soon as the first half lands. `between` emits
                extra DMAs between the halves (startup weight staging).
                """
                xt_h = [
                    xtp.tile([P, ND // 2, CH], bf16, tag="xt", name="xt")
                    for _ in range(2)
                ]
                x8 = None
                if TUNE_F8QK:
                    x8 = x8p.tile([64, ND, 2, CH], f8, tag="x8", name="x8")
                    for g in range(2):
                        nc.sync.dma_start(
                            x8[:, 4 * g : 4 * g + 4],
                            x8_d[:, c, 4 * g : 4 * g + 4],
                        )
                        if g == 0 and between is not None:
                            between()
                for g in range(2):
                    nc.sync.dma_start(
                        xt_h[g][:],
                        xt_d[
                            g * (D // 2) : (g + 1) * (D // 2),
                            c * CH : (c + 1) * CH,
                        ].rearrange("(dt p) s -> p dt s", p=P),
                    )
                    if g == 0 and between is not None and not TUNE_F8QK:
                        between()

                def xt_t(d):
                    return xt_h[d // 4][:, d % 4]

                def qk_piece(t):  # 0 = Q, 1 = K
                    cell = [None]

                    def f(drange=(0, ND), tail=True):
                        b_sb = bqk0_sb
                        pool = ytpp if (c == 0 and t == 1) else qkvp
                        tag = "ytps" if (c == 0 and t == 1) else "qv"
                        if cell[0] is None:
                            cell[0] = pool.tile(
                                [P, CH], f32, tag=tag, name=f"qk{t}"
                            )
                        ps = cell[0]
                        for d in range(*drange):
                            _lab("mm", f"qk{t}.c{c}.d{d}")
                            if TUNE_F8QK:
                                nc.tensor.matmul(
                                    ps[:],
                                    w8_sb[:, d, :, t * P : (t + 1) * P],
                                    x8[:, d],
                                    start=(d == 0),
                                    stop=(d == ND - 1),
                                    perf_mode=DR,
                                )
                            else:
                                nc.tensor.matmul(
                                    ps[:],
                                    wqk0_sb[:, d, t * P : (t + 1) * P],
                                    xt_t(d)[:, :],
                                    start=(d == 0),
                                    stop=(d == ND - 1),
                                )
                        if not tail:
                            return

                        def cvt(out_ap):
                            if TUNE_F8QK:
                                nc.vector.tensor_scalar(
                                    out=out_ap,
                                    in0=ps[:],
                                    scalar1=1.0 / F8_SCALE,
                                    scalar2=b_sb[:, t : t + 1],
                                    op0=MULT,
                                    op1=ADD,
                                )
                            else:
                                nc.vector.tensor_scalar(
                                    out=out_ap,
                                    in0=ps[:],
                                    scalar1=b_sb[:, t : t + 1],
                                    scalar2=None,
                                    op0=ADD,
                                )

                        if USE_FP8:
                            if c <= 1:
                                cvt([qt_sbs, kt_sbs][t][c][:])
                                if t == 0:
                                    # qt_drs[0..1] are never read: chunks
                                    # 0-1 run their scores in bf16
                                    return
                            st = f8p.tile([P, CH], f8, tag="f8", name="f8st")
                            cvt(st[:])
                            dr_t = [qt_drs, kt_drs][t][c]

                            def emit_dr(dr_t=dr_t, st=st):
                                # natural [h0|h1] order -> dr layout
                                for h in range(HPC):
                                    for i in range(2):
                                        nc.sync.dma_start(
                                            dr_t[32 * h : 32 * h + 32, i, :],
                                            st[
                                                64 * h + 32 * i : 64 * h
                                                + 32 * i
                                                + 32,
                                                :,
                                            ],
                                        )

                            if c <= 1:
                                # keep these off the startup HWDGE queue:
                                # first consumers are chunks 1-2
                                late_dr.append(emit_dr)
                            else:
                                emit_dr()
                        else:
                            cvt([qt_sbs, kt_sbs][t][c][:])

                    return f

                def v_piece(i0):  # s-tiles i0, i0+1
                    def f():
                        pool = ytpp if (c == 0 and i0 == 2) else qkvp
                        tag = "ytps" if (c == 0 and i0 == 2) else "qv"
                        ps = pool.tile([P, CH], f32, tag=tag, name="vps")
                        psv = ps[:].rearrange("p (i f) -> p i f", f=P)
                        for i in (i0, i0 + 1):
                            for d in range(ND):
                                _lab("mm", f"v.c{c}.i{i}.d{d}")
                                nc.tensor.matmul(
                                    psv[:, i, :],
                                    xt_t(d)[:, i * P : (i + 1) * P],
                                    wv_sb[:, d, :],
                                    start=(i == i0 and d == 0),
                                    stop=(i == i0 + 1 and d == ND - 1),
                                )
                        nc.vector.tensor_copy(
                            v_sbs[c][:, i0 : i0 + 2, :, 0:64],
                            psv[:, i0 : i0 + 2, :].rearrange(
                                "p i (h f) -> p i h f", f=HD
                            ),
                        )

                    return f

                return [qk_piece(0), qk_piece(1), v_piece(0), v_piece(2)]

            def emit_scores(j, tt, off=False):
                """QK^T for t-tile tt against s-chunk j, with causal mask.

                off=True: below-diag tile whose first SOFF columns go to a
                1-bank PSUM tile borrowed from the outproj pool (DVE-exp'd
                off the main scores ping-pong); cols [SOFF:] go through scp
                as usual (ACT-exp'd).
                """
                k = tt - 4 * j
                o = P * k if k >= 0 else 0
                sc = scp.tile([P, HPC, CH], f32, tag="sc", name="sc")
                s_off = None
                if off:
                    # head 0 -> a borrowed 1-bank psum tile (DVE-exp'd off
                    # the scores ping-pong); head 1 -> scp as usual (ACT).
                    # One DR matmul per 2KB region: two DR matmuls into the
                    # same region lock up the device.
                    s_off = qkvp.tile([P, CH], f32, tag="qv", name="sOff")
                    assert USE_FP8 and j > 1 and k < 0
                    lhsT = kt_drs[tt // 4][
                        0:32, :, (tt % 4) * P : (tt % 4 + 1) * P
                    ]
                    _lab("mm", f"scOff.j{j}.t{tt}")
                    nc.tensor.matmul(
                        s_off[:],
                        lhsT,
                        qt_drs[j][0:32, :, :],
                        start=True,
                        stop=True,
                        perf_mode=DR,
                    )
                for h in range(HPC):
                    if off and h == 0:
                        continue
                    if USE_FP8 and j <= 1:
                        # early chunks read the natural-order bf16 Q/K tiles
                        def mm(a, b, start, stop):
                            _lab("mm", f"sc.j{j}.t{tt}.h{h}")
                            nc.tensor.matmul(
                                sc[:, h, a:b],
                                kt_sbs[tt // 4][
                                    64 * h : 64 * h + 64,
                                    (tt % 4) * P : (tt % 4 + 1) * P,
                                ],
                                qt_sbs[j][64 * h : 64 * h + 64, a:b],
                                start=start,
                                stop=stop,
                            )

                    elif USE_FP8:
                        lhsT = kt_drs[tt // 4][
                            32 * h : 32 * h + 32, :, (tt % 4) * P : (tt % 4 + 1) * P
                        ]

                        def mm(a, b, start, stop):
                            _lab("mm", f"sc.j{j}.t{tt}.h{h}")
                            nc.tensor.matmul(
                                sc[:, h, a:b],
                                lhsT,
                                qt_drs[j][32 * h : 32 * h + 32, :, a:b],
                                start=start,
                                stop=stop,
                                perf_mode=DR,
                            )
                    else:
                        lhsT = kt_sbs[tt // 4][
                            64 * h : 64 * h + 64, (tt % 4) * P : (tt % 4 + 1) * P
                        ]

                        def mm(a, b, start, stop):
                            _lab("mm", f"sc.j{j}.t{tt}.h{h}")
                            nc.tensor.matmul(
                                sc[:, h, a:b],
                                lhsT,
                                qt_sbs[j][64 * h : 64 * h + 64, a:b],
                                start=start,
                                stop=stop,
                            )

                    if k < 0:
                        mm(0, CH, True, True)
                    else:
                        # one group per bank: the first pass zeroes the whole
                        # 2KB zero-region, later passes accumulate
                        if o + P < CH:
                            mm(o + P, CH, True, False)
                            mm(o, o + P, False, False)
                        else:
                            mm(o, o + P, True, False)
                        # diagonal 128-block [o:o+P] gets the ramp mask added
                        _lab("mm", f"mask.j{j}.t{tt}.h{h}")
                        nc.tensor.matmul(
                            sc[:, h, o : o + P],
                            maskA_sb[:, :],
                            maskB_sb[:, :],
                            start=False,
                            stop=True,
                        )
                return sc, o, s_off

            def emit_exp(j, tt, sc, o, s_off, late=False):
                pt = ptp.tile([P, HPC, CH], bf16, tag="pt", name="pt")
                if s_off is not None:
                    # head 0 via DVE from the borrowed bank; head 1 via ACT
                    tmp = dxp.tile([P, HPC, CH], f32, tag="dx", name="dx")
                    _lab("dve", f"expA.j{j}.t{tt}")
                    nc.vector._custom_dve(
                        exp_base_op,
                        out=tmp[:, 0, :],
                        in0=s_off[:],
                        s0=0.125 / 128.0,
                        s1=1.0 / 6.0,
                        imm2=0.5,
                    )
                    _lab("dve", f"expB.j{j}.t{tt}")
                    nc.vector._custom_dve(
                        exp_sq6_op, out=pt[:, 0, :], in0=tmp[:, 0, :]
                    )
                    nc.scalar.activation(
                        pt[:, 1:, :], sc[:, 1:, :], EXP, scale=0.125
                    )
                    return pt
                k = tt - 4 * j
                lo = max(o + P, CH - 256) if k >= 0 else CH - 256
                if late and lo < CH:
                    # kernel tail: no later scores wait on this scp buf, so
                    # DVE can exp the clean right columns straight from it
                    tmp = dxp.tile([P, HPC, CH], f32, tag="dx", name="dx")
                    _lab("dve", f"expL.j{j}.t{tt}")
                    nc.vector._custom_dve(
                        exp_base_op,
                        out=tmp[:, :, lo:],
                        in0=sc[:, :, lo:],
                        s0=0.125 / 128.0,
                        s1=1.0 / 6.0,
                        imm2=0.5,
                    )
                    nc.vector._custom_dve(
                        exp_sq6_op, out=pt[:, :, lo:], in0=tmp[:, :, lo:]
                    )
                    if o < lo:
                        nc.scalar.activation(
                            pt[:, :, o:lo], sc[:, :, o:lo], EXP, scale=0.125
                        )
                    return pt
                nc.scalar.activation(
                    pt[:, :, o:], sc[:, :, o:], EXP, scale=0.125
                )
                return pt

            def emit_av(j, tt, pt):
                k = tt - 4 * j
                vt = v_sbs[tt // 4]
                for i in range(max(k, 0), 4):
                    for h in range(HPC):
                        _lab("mm", f"av.j{j}.t{tt}.i{i}.h{h}")
                        nc.tensor.matmul(
                            avs[i // 2][:, h, i % 2, 0:65],
                            pt[:, h, i * P : (i + 1) * P],
                            vt[:, tt % 4, h, :],
                            # one accumulation group per bank (zero region):
                            # first pass zeroes it, last pass closes it
                            start=(tt == 0 and i % 2 == 0 and h == 0),
                            stop=(tt == 4 * j + i and i % 2 == 1 and h == 1),
                        )

            def emit_div(j, i, otT, tail=False):
                av = avs[i // 2]
                rc = rcp.tile([P, HPC, 1], f32, tag="rc", name="rc")
                nc.vector.reciprocal(rc[:], av[:, :, i % 2, 64:65])
                ot = otp.tile([P, P], f32, tag="ot", name="ot")
                for h in range(HPC):
                    if False:
                        nc.scalar.mul(
                            ot[:, HD * h : HD * (h + 1)],
                            av[:, h, i % 2, 0:64],
                            rc[:, h, :],
                        )
                    else:
                        nc.vector.tensor_scalar(
                            out=ot[:, HD * h : HD * (h + 1)],
                            in0=av[:, h, i % 2, 0:64],
                            scalar1=rc[:, h, :],
                            scalar2=None,
                            op0=MULT,
                        )
                # PE-transpose (f32) into av's dead region, then copy out
                # to SBUF as bf16; avoids the serialized HWDGE path
                tp = av[:, 0, i % 2, 0:P]
                _lab("mm", f"tp.j{j}.i{i}")
                nc.tensor.transpose(tp, ot[:], ident_sb[:])
                nc.vector.tensor_copy(otT[:, i, :], tp)

            def make_outproj_pieces(j, otT):
                yt_sb = ytp.tile([P, ND, CH], bf16, tag="yt", name="yt")
                rhs = otT[:].rearrange("p i f -> p (i f)")

                def piece(e):
                    def f():
                        ps = ytpp.tile([P, CH], f32, tag="ytps", name="ytps")
                        _lab("mm", f"op.j{j}.e{e}")
                        nc.tensor.matmul(
                            ps[:], wo_sb[:, e, :], rhs, start=True, stop=True
                        )
                        nc.vector.tensor_copy(yt_sb[:, e, :], ps[:])
                        if e == ND - 1:
                            nc.sync.dma_start(
                                yt_d[:, j * CH : (j + 1) * CH].rearrange(
                                    "(e p) s -> p e s", p=P
                                ),
                                yt_sb[:],
                            )

                    return f

                return [piece(e) for e in range(ND)]

            op_q = []
            otTs = {}

            def emit_div_pair(jj, pair, otT, tail=False):
                # one reciprocal for both stiles; transposes land in the
                # dead av region and copy out to SBUF in one merged move
                av = avs[pair]
                rc = rcp.tile([P, HPC, 2, 1], f32, tag="rc", name="rc")
                nc.vector.reciprocal(rc[:], av[:, :, 0:2, 64:65])
                for i in (2 * pair, 2 * pair + 1):
                    ot = otp.tile([P, P], f32, tag="ot", name="ot")
                    for h in range(HPC):
                        nc.vector.tensor_scalar(
                            out=ot[:, HD * h : HD * (h + 1)],
                            in0=av[:, h, i % 2, 0:64],
                            scalar1=rc[:, h, i % 2],
                            scalar2=None,
                            op0=MULT,
                        )
                    tp = av[:, 0, i % 2, 0:P]
                    _lab("mm", f"tp.j{jj}.i{i}")
                    nc.tensor.transpose(tp, ot[:], ident_sb[:])
                nc.vector.tensor_copy(
                    otT[:, 2 * pair : 2 * pair + 2, :], av[:, 0, 0:2, 0:P]
                )

            tail_state = {}

            def emit_tail_half(jj, half, otT):
                a, b = half * 256, half * 256 + 256
                if half == 0:
                    tail_state["yt"] = ytp.tile(
                        [P, ND, CH], bf16, tag="yt", name="yt"
                    )
                yt_sb = tail_state["yt"]
                rhs = otT[:, 2 * half : 2 * half + 2, :].rearrange(
                    "p i f -> p (i f)"
                )
                for e in range(ND):
                    if e % 4 == 0:
                        ps = ytpp.tile([P, CH], f32, tag="ytps", name="ytt")
                        ps = ps[:, 0:256]
                    elif e % 4 == 1:
                        ps = qkvp.tile([P, CH], f32, tag="qv", name="ytt")
                        ps = ps[:, 0:256]
                    else:
                        # scores pool banks are free at the tail
                        ps = scp.tile([P, HPC, CH], f32, tag="sc", name="ytt")
                        ps = ps[:, e % 2, 0:256]
                    _lab("mm", f"op.j{jj}.e{e}.h{half}")
                    nc.tensor.matmul(
                        ps, wo_sb[:, e, :], rhs, start=True, stop=True
                    )
                    if e % 2 == 0:
                        nc.scalar.copy(yt_sb[:, e, a:b], ps)
                    else:
                        nc.vector.tensor_copy(yt_sb[:, e, a:b], ps)
                    if e % 4 == 3:
                        nc.sync.dma_start(
                            yt_d[
                                (e - 3) * P : (e + 1) * P,
                                jj * CH + a : jj * CH + b,
                            ].rearrange("(e p) s -> p e s", p=P),
                            yt_sb[:, e - 3 : e + 1, a:b],
                        )

            def handle_pop(j):
                # let a DVE-exp'd tile soak: defer its AV behind the next
                # below-diag tile of the same chunk (start flag rides tt=0,
                # stop flags ride diag tiles — both excluded)
                if (
                    fifo[0][3] > 0
                    and fifo[0][1] != 0
                    and len(fifo) > 1
                    and fifo[1][0] == fifo[0][0]
                    and fifo[1][1] - 4 * fifo[1][0] < 0
                ):
                    e = fifo.pop(0)
                    e[3] -= 1
                    fifo.insert(1, e)
                jj, t0, p0_, _ = fifo.pop(0)
                emit_av(jj, t0, p0_)
                k0 = t0 - 4 * jj
                tail = jj == NCHUNK - 1
                if k0 == 1:
                    otTs[jj] = otTp.tile([P, 4, P], bf16, tag="otT", name="otT")
                    emit_div_pair(jj, 0, otTs[jj], tail=tail)
                elif k0 == 3:
                    emit_div_pair(jj, 1, otTs[jj], tail=tail)
                    if tail:
                        emit_tail_half(jj, 0, otTs[jj])
                        emit_tail_half(jj, 1, otTs.pop(jj))
                    else:
                        op_q.extend(make_outproj_pieces(jj, otTs.pop(jj)))

            # ---- merged software-pipelined stream ----
            wps = qkvp.tile([P, CH], f32, tag="qv", name="warmps")
            for w in range(TUNE_WARM):
                _lab("mm", f"warm{w}")
                nc.tensor.matmul(
                    wps[:],
                    warm[:, 0:P],
                    warm[:],
                    start=(w == 0),
                    stop=(w == TUNE_WARM - 1),
                )

            def stage_wqk_first_half():
                # wqk rows for d-tiles 0-3: all the first Q/K half needs
                if TUNE_F8QK:
                    nc.sync.dma_start(w8_sb[:, 0:4], w8_d[:, 0:4])
                else:
                    nc.sync.dma_start(
                        wqk0_sb[:, 0:4],
                        wqk0_d[0 : D // 2].rearrange(
                            "(dt p) f -> p dt f", p=P
                        ),
                    )

            p0 = make_qkv_pieces(0, between=stage_wqk_first_half)
            if TUNE_F8QK:
                nc.sync.dma_start(w8_sb[:, 4:8], w8_d[:, 4:8])
            else:
                nc.sync.dma_start(
                    wqk0_sb[:, 4:8],
                    wqk0_d[D // 2 : D].rearrange("(dt p) f -> p dt f", p=P),
                )
            nc.sync.dma_start(
                b_first_sb[:], b_first_d[:].rearrange("(c p) -> p c", p=P)
            )
            nc.sync.dma_start(
                wv_sb[:], wv_d[:].rearrange("p (dt f) -> p dt f", f=P)
            )
            qf, kf = p0[0], p0[1]
            qf(drange=(0, ND // 2), tail=False)
            kf(drange=(0, ND // 2), tail=False)
            qf(drange=(ND // 2, ND))
            kf(drange=(ND // 2, ND))
            # masks needed by the first diagonal exp (~10us in)
            nc.sync.dma_start(
                masks_sb[:], masks_d[:].rearrange("p (m f) -> p m f", f=P)
            )
            nc.sync.dma_start(ident_sb[:], ident_d[:])
            # xt(1) prefetch must beat the cold-path weight loads below
            p1 = make_qkv_pieces(1)
            # order: V01(0), Q(1), V23(0), K(1), V01(1), V23(1)
            pieces = {1: [p0[2], p1[0], p0[3], p1[1], p1[2], p1[3]]}
            nc.sync.dma_start(
                wo_sb[:], wo_d[:].rearrange("p (e f) -> p e f", f=P)
            )
            fifo = []
            dve_exp_ctr = [0]
            mj = {}
            if TUNE_DVE_MJ:
                for part in TUNE_DVE_MJ.split(","):
                    kj, vj = part.split(":")
                    mj[int(kj.lstrip("j"))] = int(vj)
            for j in range(NCHUNK):
                ntt = 4 * j + 4
                cur = pieces.pop(j + 1, [])
                for tt in range(ntt):
                    k = tt - 4 * j
                    lt = (ntt - tt) if j == NCHUNK - 1 else 999
                    m_here = mj.get(j, 0) if mj else (
                        TUNE_DVE_M
                        if TUNE_DVE_M > 0 and j > max(2, TUNE_DVE_JMIN - 1)
                        else 0
                    )
                    off = (
                        m_here > 0
                        and k < 0  # below-diag: no -1e30 masked entries
                        and USE_FP8
                        and lt > TUNE_DVE_TAIL
                    )
                    if off:
                        dve_exp_ctr[0] += 1
                        off = dve_exp_ctr[0] % m_here == 0
                    late = (
                        TUNE_LATE > 0
                        and j == NCHUNK - 1
                        and lt <= TUNE_DVE_TAIL
                    )
                    sc, o, s_off = emit_scores(j, tt, off=off)
                    pt = emit_exp(j, tt, sc, o, s_off, late=late)
                    fifo.append([j, tt, pt, TUNE_DVE_ROT if off else 0])
                    if tt >= TUNE_PIECE_TT and cur:
                        cur.pop(0)()
                        if j == 0 and cur:
                            cur.pop(0)()
                    if j in (1, 2) and late_dr:
                        late_dr.pop(0)()
                    if len(fifo) > (4 if j == 0 else TUNE_LAG):
                        handle_pop(j)
                    if tt == max(TUNE_MK, ntt // 2) and j + 2 < NCHUNK:
                        pieces[j + 2] = make_qkv_pieces(j + 2)
                    if tt >= TUNE_OP_TT and op_q:
                        op_q.pop(0)()
                if cur and j + 2 < NCHUNK:
                    # carry leftovers into the next chunk's piece stream so
                    # they don't lump up right before its first scores
                    pieces[j + 2] = cur + pieces.get(j + 2, [])
                else:
                    while cur:
                        cur.pop(0)()
            while fifo:
                handle_pop(NCHUNK - 1)
            while op_q:
                op_q.pop(0)()

    return nc


@functools.lru_cache(maxsize=1)
def _get_nc(S_arg=S):
    nc = build_nc()
    nc.compile()
    return nc


def _dr_pack(a):
    """[D, F] -> DoubleRow-packed [64, ND, 2, F]: row d = dt*128 + i*64 + q
    sits at [q, dt, i, :] (partitions 0-63 only)."""
    F = a.shape[1]
    return np.ascontiguousarray(
        a.reshape(ND, 2, 64, F).transpose(2, 0, 1, 3)
    )


def make_in_maps(input, Wqkv, bqkv, Wo):
    F8 = ml_dtypes.float8_e4m3fn
    x = np.asarray(input, dtype=np.float32).reshape(S, D)
    xt = np.ascontiguousarray(x.T).astype(BF16)
    Wqkv = np.asarray(Wqkv, dtype=np.float32)
    bqkv = np.asarray(bqkv, dtype=np.float32)
    Wo = np.asarray(Wo, dtype=np.float32)
    Wq, Wk, Wv = Wqkv[:, 0:D], Wqkv[:, D : 2 * D], Wqkv[:, 2 * D : 3 * D]
    bq, bk = bqkv[0:D], bqkv[D : 2 * D]
    if TUNE_F8QK:
        # x8: [64, NCHUNK, ND, 2, CH] chunk-major DR packing of X^T fp8
        x8_full = _dr_pack(np.ascontiguousarray(x.T).astype(F8))  # [64,8,2,S]
        x8 = np.ascontiguousarray(
            x8_full.reshape(64, ND, 2, NCHUNK, CH).transpose(0, 3, 1, 2, 4)
        )

    r = np.arange(P)
    maskA = np.where(r[:, None] < r[None, :], np.float32(-1e30), np.float32(0))
    maskB = (r[:, None] >= r[None, :]).astype(np.float32)
    maskA = np.ascontiguousarray(maskA.astype(BF16))
    maskB = np.ascontiguousarray(maskB.astype(BF16))
    ident = np.ascontiguousarray(np.eye(P, dtype=np.float32))

    in_maps = []
    for c in range(NCORES):
        hs = [HPC * c + i for i in range(HPC)]

        def headcols(W):
            return np.concatenate([W[:, h * HD : (h + 1) * HD] for h in hs], 1)

        def headvec(b):
            return np.concatenate([b[h * HD : (h + 1) * HD] for h in hs], 0)

        wq0, wk0 = headcols(Wq), headcols(Wk)
        bq0, bk0 = headvec(bq), headvec(bk)
        extra = {}
        if TUNE_F8QK:
            w8 = np.concatenate([wq0, wk0], axis=1) * np.float32(F8_SCALE)
            extra["x8"] = x8
            extra["w8"] = _dr_pack(w8.astype(F8))
        in_maps.append(
            {
                **extra,
                "xt": xt,
                "wqk0": np.ascontiguousarray(
                    np.concatenate([wq0, wk0], axis=1).astype(BF16)
                ),
                "bqk0": np.ascontiguousarray(
                    np.concatenate([bq0, bk0], axis=0).astype(np.float32)
                ),
                "wv": np.ascontiguousarray(
                    headcols(Wv)
                    .astype(BF16)
                    .reshape(ND, P, P)
                    .transpose(1, 0, 2)
                    .reshape(P, ND * P)
                ),
                "wo": np.ascontiguousarray(
                    Wo[hs[0] * HD : hs[0] * HD + HPC * HD, :].astype(BF16)
                ),
                "masks": np.ascontiguousarray(
                    np.concatenate([maskA, maskB], axis=1)
                ),
                "ident": ident,
            }
        )
    return in_maps


def kernel(input, Wqkv, bqkv, Wo, bo):
    from concourse.bass_utils import run_bass_kernel_spmd

    nc = _get_nc()
    in_maps = make_in_maps(input, Wqkv, bqkv, Wo)
    res = None
    last_exc = None
    for _attempt in range(3):  # transient NRT/device errors: retry
        try:
            res = run_bass_kernel_spmd(nc, in_maps, core_ids=list(range(NCORES)))
            break
        except Exception as e:  # noqa: BLE001
            last_exc = e
    if res is None:
        raise last_exc
    acc = np.zeros((D, S), np.float32)
    for r in res.results:
        acc += np.asarray(r["yt"], dtype=np.float32)
    y = np.ascontiguousarray(acc.T)
    bv = np.asarray(bqkv, np.float32)[2 * D : 3 * D]
    y += (bv @ np.asarray(Wo, np.float32) + np.asarray(bo, np.float32))[None, :]
    return y.reshape(1, S, D)

